# revision 46
# baseline (speedup 1.0000x reference)
"""Trainium2 Bass kernel for nn_CrossAttention_77240691851613.

Reference (B=2, L=2048, D=64, H=8, OUT=720), per core (batch b, 2 heads):
    q = x_q @ Wq          k = x_k @ Wk + bk      v = x_v @ Wv
    S^T[l,k] = q^T . k^T  (contraction d=64)     P = exp(S^T/8)
    out[k,d] = sum_l P[l,k] V[l,d] / Z[k],  Z = sum_l P
    F = mean_h(out)^T @ Wl  (+ biases on host)

Design (sharding: 8 cores = 2 batches x 4 head-groups of 2 heads):
 - Inputs ship as 3 blob DMAs (wqk+xq/xk chunk0 head, rest, vt+wv);
   each extra dma_start costs ~625ns HWDGE + 900ns sem serially at
   startup, and the head blob lets proj->copy->score->exp start ~2us
   earlier. wl (2.8MB) loads mid-stream.
 - q/k projections in fp8e4 + DoubleRow (0.5 cyc/row); bk is folded in
   as a 33rd contraction row (fp8 bias/ones row). Scores are bf16.
 - exp of 8.4M scores is the wall: [128,1024] lt-pair tiles alternate
   strictly between ACT (true exp -> fp8e4 out, 1038ns) and DVE
   (Schraudolph bit trick: i8 = round(S*1.4427) + 56 IS the e4m3 bit
   pattern of exp(S/8); +-4% rel err averages out under softmax).
   GPSIMD cannot read PSUM on HW, so only these 2 engines. The st ring
   (bufs=3) locksteps the pair cadence at ~1.34us (ring-lag ~900ns /
   3 bufs); strict AD alternation measured best by scan.
 - et layout [128, h, ltp, kt, j2, 128] keeps every exp write and
   every PV read a tight contiguous box (no phantom overlap deps).
 - PV flipped + fp8 DoubleRow over l-tile pairs: stationary et
   [128,2,128], moving V [128,2,64] -> out [k=128,64]; Z via 1-col
   ones matmuls into pv[:,4,:] (own group). Per kt-pair combine:
   DVE reciprocal, then ACT copies pv->SBUF f32 and the otherwise-idle
   GPSIMD/Pool does the broadcast multiply (pv[j] * rz[j]) + strided
   pair-add into m (takes ~6us of combine work off the two hot
   engines); the tail ktps 6,7 stay on DVE (free at that point).
   NOTE: the TileScheduler orders by deps, not emission order.
 - final projection flipped: out [720-block, 64], moving = m (64
   cols), all 6 blocks accumulating in ONE psum bank with a SINGLE
   start (a second start in the same bank re-pends the whole-bank
   zero region and wipes earlier partial sums - cost 4e-3 absmax).
 - single output DMA; fo copies split ACT/DVE.
 - bq cancels in softmax over l; bv and the head-mean 1/8 are applied
   on the host gather path.
"""

import numpy as np

B = 2
L = 2048
D = 64
H = 8
OUT = 720
P = 128
KC = 512  # score k-chunk (one PSUM bank)
NLT = 16
N_CORES = 8

# e4m3-bit-trick exp constants: i8 = round(S * (0.125*8*log2e)) + 56
SCH_A = 1.4426950408889634
SCH_B = 56.0

_PROGRAM_CACHE = {}


def build_program():
    if "nc" in _PROGRAM_CACHE:
        return _PROGRAM_CACHE["nc"]

    from contextlib import ExitStack

    import concourse.bass as bass
    import concourse.tile as tile
    from concourse import bacc, mybir

    dt = mybir.dt
    f32 = dt.float32
    bf16 = dt.bfloat16
    f8 = dt.float8e4
    i8 = dt.int8
    AF = mybir.ActivationFunctionType
    ALU = mybir.AluOpType
    DR = mybir.MatmulPerfMode.DoubleRow
    ts = bass.ts
    ds = bass.ds

    nc = bacc.Bacc("TRN2", target_bir_lowering=False, debug=False,
                   num_devices=N_CORES)

    # ---- DRAM I/O --------------------------------------------------------
    # qkx blob [33, 2(j), 256 wqk | 512 xq_c0 | 512 xk_c0 | 1536 xq_c1-3
    # | 1536 xk_c1-3] f8. Two DMAs: a small head (wqk + chunk 0 of q/k)
    # lands ~1.6us so the first proj->copy->score->exp chain starts ~3us
    # earlier than one monolithic load; the rest follows right behind.
    qkx_t = nc.dram_tensor("qkx", [33, 2, 4352], f8, kind="ExternalInput").ap()
    # vtw blob [64, 2048 vt | 128 wv] bf16.
    vtw_t = nc.dram_tensor("vtw", [D, L + P], bf16, kind="ExternalInput").ap()
    wl_t = nc.dram_tensor("wl_t", [P, NLT, OUT], bf16, kind="ExternalInput").ap()
    f_out = nc.dram_tensor("f_out", [P, 6, D], f32, kind="ExternalOutput").ap()

    with tile.TileContext(nc) as tc, ExitStack() as ctx:
        const = ctx.enter_context(tc.tile_pool(name="const", bufs=1))

        # ---- SBUF persistent tiles --------------------------------------
        qkx_sb = const.tile([33, 2, 4352], f8, tag="qkx")
        nc.sync.dma_start(qkx_sb[:, :, 0:1280], qkx_t[:, :, 0:1280])
        nc.sync.dma_start(qkx_sb[:, :, ds(1280, 3072)],
                          qkx_t[:, :, ds(1280, 3072)])
        vtw_sb = const.tile([D, L + P], bf16, tag="vtw")
        nc.sync.dma_start(vtw_sb[:], vtw_t)
        wqk_sb = qkx_sb[:, :, 0:256]          # [33, 2, 2*128]: qk via ds()
        vt_sb = vtw_sb[:, 0:L]
        wv_sb = vtw_sb[:, ds(L, P)]

        def x_chunk(qk, ch):
            # 512-col l-chunk of xq (qk=0) / xk (qk=1) in the blob
            off = 256 + 512 * qk if ch == 0 else 1280 + 1536 * qk + 512 * (ch - 1)
            return qkx_sb[:, :, ds(off, 512)]

        # q/k [128(h,d), L] bf16, filled chunkwise by f32->bf16 copies
        # spread across engines (the only transport PSUM allows).
        qf = const.tile([P, L], bf16, tag="qf")
        kf = const.tile([P, L], bf16, tag="kf")
        # exp tile [P, h, ltp, kt(16), j2(2), 128]: PV lhsT (h,ltp,kt) reads
        # the contiguous 256B window; exp writes (h,lt,kc) cover 4 kt
        # sub-windows of one j2 — boxes never span other k-quarters.
        et = const.tile([P, 2, 8, NLT, 2, P], f8, tag="et")
        et_i8 = et.bitcast(i8)
        v2 = const.tile([P, 8, 2, 2, D], f8, tag="v2")  # (ltp, j2, h, d)
        ones2 = const.tile([P, 2, 1], f8, tag="ones2")
        rz = const.tile([P, 32], f32, tag="rz")         # 1/Z, col=(kt,h)
        m_sb = const.tile([P, NLT, D], bf16, tag="m")   # combined heads
        fo_sb = const.tile([P, 6 * D], f32, tag="fo")

        # ACT exp-table warmup
        warm = const.tile([1, 8], f32, tag="warm")
        nc.vector.memset(warm[:], 0.0)
        nc.scalar.activation(warm[:], warm[:], AF.Exp)
        nc.gpsimd.memset(ones2[:], 1.0)

        # Greedy (projected-finish) engine-assignment bookkeeping: every
        # elementwise op injects its modeled busy cost (TimelineSim: ACT
        # 185ns + 0.833/col, DVE 125ns + 1.042/col) into its engine total.
        state = {"busy": {"A": 0.0, "D": 0.0}, "pv_tiles": {}}

        def _inject_later(eng, ns):
            state["busy"][eng] += ns

        # ---- Phase P: q/k projections (fp8 DoubleRow, bias via 33rd
        # contraction row) -> PSUM f32 -> DMA straight into SBUF. No
        # element-wise conversion pass at all.
        stp_cm = tc.tile_pool(name="st", bufs=2, space="PSUM")
        stp = stp_cm.__enter__()  # closed explicitly before the final pool
        pp_cm = tc.tile_pool(name="proj_psum", bufs=1, space="PSUM")
        pp = pp_cm.__enter__()

        def qk_proj(qk, ch, eng):
            dst = qf if qk == 0 else kf
            ps = pp.tile([P, 512], f32, tag=f"ps{qk}", name=f"ps{qk}")
            nc.tensor.matmul(
                ps[:], wqk_sb[:, :, ds(128 * qk, P)], x_chunk(qk, ch),
                start=True, stop=True, perf_mode=DR)
            if eng == "A":
                nc.scalar.copy(dst[:, ts(ch, 512)], ps[:])
                _inject_later("A", 612)
            else:
                nc.vector.tensor_copy(dst[:, ts(ch, 512)], ps[:])
                _inject_later("D", 658)

        for (qk, ch, eng) in ((0, 0, "A"), (1, 0, "D"), (0, 1, "A"),
                              (0, 2, "D"), (0, 3, "A"), (1, 1, "D"),
                              (1, 2, "A"), (1, 3, "D")):
            qk_proj(qk, ch, eng)

        wl_sb = const.tile([P, NLT, OUT], bf16, tag="wl")
        nc.sync.dma_start(wl_sb[:], wl_t)

        def emit_vproj():
            # 4 l-tiles of V per [128,512] proj-pool tile (shared-bank
            # accumulation groups: one start covers the tile), one big
            # f32->fp8 copy each on ACT/DVE.
            for c4 in range(4):
                ps = pp.tile([P, 4, P], f32, tag=f"ps{c4 % 2}",
                             name="psv")
                for i in range(4):
                    nc.tensor.matmul(ps[:, i, :],
                                     vt_sb[:, ds(512 * c4 + 128 * i, P)],
                                     wv_sb[:],
                                     start=(i == 0), stop=(i == 3))
                dst = v2[:, ds(2 * c4, 2), :, :, :]
                if c4 % 2 == 0:
                    nc.scalar.copy(dst, ps[:])
                    _inject_later("A", 612)
                else:
                    nc.vector.tensor_copy(dst, ps[:])
                    _inject_later("D", 658)

        # ---- main pools --------------------------------------------------
        inject = _inject_later

        import os
        exppat = os.environ.get("EXPPAT", "AD" * 30 + "ADDA")
        state["tile_no"] = 0

        def pick_exp_engine():
            # Pattern override (tuning knob); else projected-finish greedy
            # constrained to never run >2 consecutive tiles on one engine
            # (the in-order PSUM ring needs a near-alternating cadence).
            n = state["tile_no"]
            state["tile_no"] = n + 1
            if exppat:
                eng = exppat[n % len(exppat)]
            else:
                a = state["busy"]["A"] + 1038.0
                d = state["busy"]["D"] + 1192.0
                eng = "A" if a <= d else "D"
                last2 = state.get("last2", "")
                if last2 == eng * 2:
                    eng = "D" if eng == "A" else "A"
                state["last2"] = (state.get("last2", "") + eng)[-2:]
            inject(eng, 1038.0 if eng == "A" else 1192.0)
            return eng

        def emit_scores(h, kcs, fillers=()):
            """per lt-pair (one engine): two score mms + ONE 1024-wide exp.
            fillers: PE-op callables interleaved between tiles to keep score
            production cadence even (B-phase mms would otherwise pause it)."""
            fillers = list(fillers)
            for kc in kcs:
                for ltp in range(8):
                    eng = pick_exp_engine()
                    st = stp.tile([P, 2, 4, P], f32, tag="st", name="st",
                                  bufs=3)
                    for j2 in range(2):
                        nc.tensor.matmul(
                            st[:, j2, :, :],
                            qf[ds(64 * h, 64), ts(2 * ltp + j2, P)],
                            kf[ds(64 * h, 64), ts(kc, KC)],
                            start=True, stop=True)
                    stv = st[:, :, :, :].transpose((0, 2, 1, 3))
                    out = et[:, h, ltp, ds(4 * kc, 4), :, :]
                    if eng == "A":
                        nc.scalar.activation(out, stv, AF.Exp, scale=0.125)
                    else:
                        nc.vector.tensor_scalar(
                            et_i8[:, h, ltp, ds(4 * kc, 4), :, :],
                            stv, SCH_A, SCH_B, op0=ALU.mult, op1=ALU.add)
                    if fillers:
                        f = fillers.pop(0)
                        if f is not None:
                            f()
            while fillers:
                f = fillers.pop(0)
                if f is not None:
                    f()

        def emit_z_mms(ktp):
            """Z matmuls (own accumulation group over pv[:,4,:])."""
            pv = state["pvp"].tile([P, 5, D], f32, tag="pv", name="pv")
            state["pv_tiles"][ktp] = pv
            for i in range(2):
                kt = 2 * ktp + i
                for h in range(2):
                    for ltp in range(8):
                        nc.tensor.matmul(
                            pv[:, 4, ds(2 * i + h, 1)],
                            et[:, h, ltp, kt, :, :],
                            ones2[:],
                            start=(i == 0 and h == 0 and ltp == 0),
                            stop=(i == 1 and h == 1 and ltp == 7),
                            perf_mode=DR)

        def emit_pv_mms(ktp):
            """PV matmuls (own group over pv[:,0:4,:])."""
            pv = state["pv_tiles"][ktp]
            for i in range(2):
                kt = 2 * ktp + i
                for h in range(2):
                    for ltp in range(8):
                        nc.tensor.matmul(
                            pv[:, 2 * i + h, :],
                            et[:, h, ltp, kt, :, :],
                            v2[:, ltp, :, h, :],
                            start=(i == 0 and h == 0 and ltp == 0),
                            stop=(i == 1 and h == 1 and ltp == 7),
                            perf_mode=DR)

        cmb_mode = os.environ.get("CMB", "pool")

        def emit_combines(ktp, mpool):
            # Per-ktp normalize + head-combine. Modes:
            #  dve:  recip/mult/add on DVE straight from PSUM (1x rate).
            #  pool: ACT copies pv->SBUF, Pool does mult/add.
            #  sbuf: ACT copies the WHOLE pv tile (PV+Z) -> SBUF bf16; DVE
            #        then runs recip/mult/add all-SBUF, where its 2x mode
            #        applies (~420ns/ktp vs ~780 PSUM-direct), and pv's
            #        PSUM bank frees at the copy.
            pv = state["pv_tiles"].pop(ktp)
            tmp = mpool.tile([P, 2, 2, D], f32, tag="tmp")
            rzb = rz[:, ds(4 * ktp, 4)].broadcast_to([P, 4, D])
            if cmb_mode == "sbuf":
                pvc = mpool.tile([P, 5, D], f32, tag="pvc")
                nc.scalar.copy(pvc[:], pv[:])
                inject("A", 452)
                nc.vector.reciprocal(rz[:, ds(4 * ktp, 4)], pvc[:, 4, 0:4])
                inject("D", 100)
                nc.vector.tensor_tensor(tmp[:], pvc[:, 0:4, :], rzb,
                                        op=ALU.mult)
                inject("D", 195)
                nc.vector.tensor_add(
                    m_sb[:, ds(2 * ktp, 2), :], tmp[:, :, 0, :],
                    tmp[:, :, 1, :])
                inject("D", 96)
                return
            nc.vector.reciprocal(rz[:, ds(4 * ktp, 4)], pv[:, 4, 0:4])
            inject("D", 129)
            if cmb_mode == "pool" and ktp < 6:
                pvc = mpool.tile([P, 2, 2, D], f32, tag="pvc")
                nc.scalar.copy(pvc[:], pv[:, 0:4, :])
                inject("A", 398)
                nc.gpsimd.tensor_tensor(tmp[:], pvc[:], rzb, op=ALU.mult)
                nc.gpsimd.tensor_add(
                    m_sb[:, ds(2 * ktp, 2), :], tmp[:, :, 0, :],
                    tmp[:, :, 1, :])
            else:
                nc.vector.tensor_tensor(tmp[:], pv[:, 0:4, :], rzb,
                                        op=ALU.mult)
                inject("D", 392)
                nc.vector.tensor_add(
                    m_sb[:, ds(2 * ktp, 2), :], tmp[:, :, 0, :],
                    tmp[:, :, 1, :])
                inject("D", 258)

        mpool = ctx.enter_context(tc.tile_pool(name="mp", bufs=2))

        # ---- pipeline ----------------------------------------------------
        emit_scores(0, [0])
        emit_vproj()
        emit_scores(1, [0])
        pp_cm.__exit__(None, None, None)

        pvp_cm = tc.tile_pool(name="pv", bufs=2, space="PSUM")
        state["pvp"] = pvp_cm.__enter__()

        emit_scores(0, [1])
        emit_scores(1, [1])

        # The TileScheduler orders by dependencies, not emission order, so
        # B/C work is simply emitted after each quarter's score blocks.
        emit_scores(0, [2])
        emit_scores(1, [2])
        for ktp in (0, 1):
            emit_z_mms(ktp)
            emit_pv_mms(ktp)
            emit_combines(ktp, mpool)
        emit_scores(0, [3])
        for ktp in (2, 3):
            emit_z_mms(ktp)
            emit_pv_mms(ktp)
            emit_combines(ktp, mpool)
        emit_scores(1, [3])
        for ktp in (4, 5, 6, 7):
            emit_z_mms(ktp)
            emit_pv_mms(ktp)
            emit_combines(ktp, mpool)

        pvp_cm.__exit__(None, None, None)
        stp_cm.__exit__(None, None, None)

        with tc.tile_pool(name="fop", bufs=1, space="PSUM") as fop:
            fo = fop.tile([P, 6 * D], f32, tag="fo", name="fo")
            # ONE accumulation group: a single start pends the whole-bank
            # zero region (a second start inside the bank re-pends it and
            # WIPES earlier groups' partial sums — measured 4e-3 absmax).
            for kt in range(NLT):
                for ob in range(6):
                    w = min(P, OUT - 128 * ob)
                    nc.tensor.matmul(
                        fo[0:w, ds(64 * ob, 64)],
                        wl_sb[:, kt, ds(128 * ob, w)],
                        m_sb[:, kt, :],
                        start=(kt == 0 and ob == 0),
                        stop=(kt == NLT - 1 and ob == 5))
            # copies split across engines; ONE output DMA (each extra DMA
            # costs a serial 625ns HWDGE slot + 900ns sem at the very end).
            nc.scalar.copy(fo_sb[:, 0:192], fo[:, 0:192])
            nc.vector.tensor_copy(fo_sb[:, 192:384], fo[:, 192:384])
            nc.sync.dma_start(f_out[:], fo_sb[:])

    nc.compile()
    _PROGRAM_CACHE["nc"] = nc
    return nc


def prep_in_maps(query, key, value, Wq, Wk, bk, Wv, Wl):
    """Host-side shard + layout prep: one in_map per core."""
    import ml_dtypes

    F8 = ml_dtypes.float8_e4m3
    BF = ml_dtypes.bfloat16

    query = np.asarray(query, np.float32)
    key = np.asarray(key, np.float32)
    value = np.asarray(value, np.float32)
    Wq = np.asarray(Wq, np.float32)
    Wk = np.asarray(Wk, np.float32)
    bk = np.asarray(bk, np.float32)
    Wv = np.asarray(Wv, np.float32)
    Wl = np.asarray(Wl, np.float32)

    wl_prep = np.ascontiguousarray(
        Wl.reshape(NLT, P, OUT).transpose(1, 0, 2).astype(BF))

    def dsplit33(xt, ones_row):
        # [64, L] -> [33, 2, L]: d = 32j + p for p<32; row 32 = (ones, 0)
        out = np.zeros((33, 2, xt.shape[1]), np.float32)
        out[:32] = xt.reshape(2, 32, -1).transpose(1, 0, 2)
        out[32, 0] = ones_row
        return np.ascontiguousarray(out)

    in_maps = []
    for c in range(N_CORES):
        b, g = divmod(c, 4)
        sl = slice(P * g, P * (g + 1))
        # wqk [33(p), 2(j), 2(qk), 128(o)]: row 32 j=0 = bias (k only)
        wqk = np.zeros((33, 2, 2, P), np.float32)
        wqk[:32, :, 0, :] = Wq[:, sl].reshape(2, 32, P).transpose(1, 0, 2)
        wqk[:32, :, 1, :] = Wk[:, sl].reshape(2, 32, P).transpose(1, 0, 2)
        wqk[32, 0, 1, :] = bk[sl]
        # qkx blob [33, 2, 256 wqk | 512 xq_c0 | 512 xk_c0 | 1536 xq_c1-3
        # | 1536 xk_c1-3]
        xq33 = dsplit33(query[b].T, 1.0).astype(F8)
        xk33 = dsplit33(key[b].T, 1.0).astype(F8)
        qkx = np.empty((33, 2, 4352), F8)
        qkx[:, :, 0:256] = wqk.astype(F8).reshape(33, 2, 256)
        qkx[:, :, 256:768] = xq33[:, :, 0:512]
        qkx[:, :, 768:1280] = xk33[:, :, 0:512]
        qkx[:, :, 1280:2816] = xq33[:, :, 512:]
        qkx[:, :, 2816:4352] = xk33[:, :, 512:]
        vtw = np.empty((D, L + P), BF)
        vtw[:, :L] = value[b].T.astype(BF)
        vtw[:, L:] = Wv[:, sl].astype(BF)
        in_maps.append({
            "qkx": qkx,
            "vtw": np.ascontiguousarray(vtw),
            "wl_t": wl_prep,
        })
    return in_maps


def combine_outputs(f_outs, bv, Wl, bl):
    """Host-side gather: per-core F^T blocks -> full output + biases."""
    bv = np.asarray(bv, np.float32)
    Wl = np.asarray(Wl, np.float32)
    bl = np.asarray(bl, np.float32)
    F = np.stack(f_outs).astype(np.float32)        # [8, 128, 6, 64]
    # [core, p, ob, d] -> [core, d, 128*ob + p] -> F [core, 64, 720]
    Fc = F.transpose(0, 3, 2, 1).reshape(N_CORES, D, 6 * P)[:, :, :OUT]
    out = np.empty((B, D, OUT), np.float32)
    for b in range(B):
        out[b] = 0.125 * Fc[4 * b:4 * b + 4].sum(axis=0)
    bv_mean = bv.reshape(H, D).mean(axis=0)
    out += bv_mean[None, :, None] * Wl.sum(axis=0)[None, None, :]
    out += bl[None, None, :]
    return out


def kernel(query, key, value, Wq, bq, Wk, bk, Wv, bv, Wl, bl):
    from concourse.bass_utils import run_bass_kernel_spmd

    nc = build_program()
    in_maps = prep_in_maps(query, key, value, Wq, Wk, bk, Wv, Wl)
    res = run_bass_kernel_spmd(nc, in_maps, core_ids=list(range(N_CORES)))
    f_outs = [res.results[c]["f_out"] for c in range(N_CORES)]
    return combine_outputs(f_outs, bv, Wl, bl)



# revision 49
# speedup vs baseline: 1.0007x; 1.0007x over previous
"""Trainium2 Bass kernel for nn_CrossAttention_77240691851613.

Reference (B=2, L=2048, D=64, H=8, OUT=720), per core (batch b, 2 heads):
    q = x_q @ Wq          k = x_k @ Wk + bk      v = x_v @ Wv
    S^T[l,k] = q^T . k^T  (contraction d=64)     P = exp(S^T/8)
    out[k,d] = sum_l P[l,k] V[l,d] / Z[k],  Z = sum_l P
    F = mean_h(out)^T @ Wl  (+ biases on host)

Design (sharding: 8 cores = 2 batches x 4 head-groups of 2 heads):
 - Inputs ship as 3 blob DMAs (wqk+xq/xk chunk0 head, rest, vt+wv);
   each extra dma_start costs ~625ns HWDGE + 900ns sem serially at
   startup, and the head blob lets proj->copy->score->exp start ~2us
   earlier. wl (2.8MB) loads mid-stream.
 - q/k projections in fp8e4 + DoubleRow (0.5 cyc/row); bk is folded in
   as a 33rd contraction row (fp8 bias/ones row). Scores are bf16.
 - exp of 8.4M scores is the wall: [128,1024] lt-pair tiles alternate
   strictly between ACT (true exp -> fp8e4 out, 1038ns) and DVE
   (Schraudolph bit trick: i8 = round(S*1.4427) + 56 IS the e4m3 bit
   pattern of exp(S/8); +-4% rel err averages out under softmax).
   GPSIMD cannot read PSUM on HW, so only these 2 engines. The st ring
   (bufs=3) locksteps the pair cadence at ~1.34us (ring-lag ~900ns /
   3 bufs); strict AD alternation measured best by scan.
 - et layout [128, h, ltp, kt, j2, 128] keeps every exp write and
   every PV read a tight contiguous box (no phantom overlap deps).
 - PV flipped + fp8 DoubleRow over l-tile pairs: stationary et
   [128,2,128], moving V [128,2,64] -> out [k=128,64]; Z via 1-col
   ones matmuls into pv[:,4,:] (own group). Per kt-pair combine:
   DVE reciprocal, then ACT copies pv->SBUF f32 and the otherwise-idle
   GPSIMD/Pool does the broadcast multiply (pv[j] * rz[j]) + strided
   pair-add into m (takes ~6us of combine work off the two hot
   engines); the tail ktps 6,7 stay on DVE (free at that point).
   NOTE: the TileScheduler orders by deps, not emission order.
 - final projection flipped: out [720-block, 64], moving = m (64
   cols), all 6 blocks accumulating in ONE psum bank with a SINGLE
   start (a second start in the same bank re-pends the whole-bank
   zero region and wipes earlier partial sums - cost 4e-3 absmax).
 - single output DMA; fo copies split ACT/DVE.
 - bq cancels in softmax over l; bv and the head-mean 1/8 are applied
   on the host gather path.
"""

import numpy as np

B = 2
L = 2048
D = 64
H = 8
OUT = 720
P = 128
KC = 512  # score k-chunk (one PSUM bank)
NLT = 16
N_CORES = 8

# e4m3-bit-trick exp constants: i8 = round(S * (0.125*8*log2e)) + 56
SCH_A = 1.4426950408889634
SCH_B = 56.0

_PROGRAM_CACHE = {}


def build_program():
    if "nc" in _PROGRAM_CACHE:
        return _PROGRAM_CACHE["nc"]

    from contextlib import ExitStack

    import concourse.bass as bass
    import concourse.tile as tile
    from concourse import bacc, mybir

    dt = mybir.dt
    f32 = dt.float32
    bf16 = dt.bfloat16
    f8 = dt.float8e4
    i8 = dt.int8
    AF = mybir.ActivationFunctionType
    ALU = mybir.AluOpType
    DR = mybir.MatmulPerfMode.DoubleRow
    ts = bass.ts
    ds = bass.ds

    nc = bacc.Bacc("TRN2", target_bir_lowering=False, debug=False,
                   num_devices=N_CORES)

    # ---- DRAM I/O --------------------------------------------------------
    # qkx blob [33, 2(j), 256 wqk | 512 xq_c0 | 512 xk_c0 | 1536 xq_c1-3
    # | 1536 xk_c1-3] f8. Two DMAs: a small head (wqk + chunk 0 of q/k)
    # lands ~1.6us so the first proj->copy->score->exp chain starts ~3us
    # earlier than one monolithic load; the rest follows right behind.
    qkx_t = nc.dram_tensor("qkx", [33, 2, 4352], f8, kind="ExternalInput").ap()
    # vtw blob [64, 2048 vt | 128 wv] bf16.
    vtw_t = nc.dram_tensor("vtw", [D, L + P], bf16, kind="ExternalInput").ap()
    wl_t = nc.dram_tensor("wl_t", [P, NLT, OUT], bf16, kind="ExternalInput").ap()
    f_out = nc.dram_tensor("f_out", [P, 6, D], f32, kind="ExternalOutput").ap()

    with tile.TileContext(nc) as tc, ExitStack() as ctx:
        const = ctx.enter_context(tc.tile_pool(name="const", bufs=1))

        # ---- SBUF persistent tiles --------------------------------------
        qkx_sb = const.tile([33, 2, 4352], f8, tag="qkx")
        nc.sync.dma_start(qkx_sb[:, :, 0:1280], qkx_t[:, :, 0:1280])
        nc.sync.dma_start(qkx_sb[:, :, ds(1280, 3072)],
                          qkx_t[:, :, ds(1280, 3072)])
        vtw_sb = const.tile([D, L + P], bf16, tag="vtw")
        nc.sync.dma_start(vtw_sb[:], vtw_t)
        wqk_sb = qkx_sb[:, :, 0:256]          # [33, 2, 2*128]: qk via ds()
        vt_sb = vtw_sb[:, 0:L]
        wv_sb = vtw_sb[:, ds(L, P)]

        def x_chunk(qk, ch):
            # 512-col l-chunk of xq (qk=0) / xk (qk=1) in the blob
            off = 256 + 512 * qk if ch == 0 else 1280 + 1536 * qk + 512 * (ch - 1)
            return qkx_sb[:, :, ds(off, 512)]

        # q/k [128(h,d), L] bf16, filled chunkwise by f32->bf16 copies
        # spread across engines (the only transport PSUM allows).
        qf = const.tile([P, L], bf16, tag="qf")
        kf = const.tile([P, L], bf16, tag="kf")
        # exp tile [P, h, ltp, kt(16), j2(2), 128]: PV lhsT (h,ltp,kt) reads
        # the contiguous 256B window; exp writes (h,lt,kc) cover 4 kt
        # sub-windows of one j2 — boxes never span other k-quarters.
        et = const.tile([P, 2, 8, NLT, 2, P], f8, tag="et")
        et_i8 = et.bitcast(i8)
        v2 = const.tile([P, 8, 2, 2, D], f8, tag="v2")  # (ltp, j2, h, d)
        ones2 = const.tile([P, 2, 1], f8, tag="ones2")
        rz = const.tile([P, 32], f32, tag="rz")         # 1/Z, col=(kt,h)
        m_sb = const.tile([P, NLT, D], bf16, tag="m")   # combined heads
        fo_sb = const.tile([P, 6 * D], f32, tag="fo")

        # ACT exp-table warmup
        warm = const.tile([1, 8], f32, tag="warm")
        nc.vector.memset(warm[:], 0.0)
        nc.scalar.activation(warm[:], warm[:], AF.Exp)
        nc.gpsimd.memset(ones2[:], 1.0)

        # Greedy (projected-finish) engine-assignment bookkeeping: every
        # elementwise op injects its modeled busy cost (TimelineSim: ACT
        # 185ns + 0.833/col, DVE 125ns + 1.042/col) into its engine total.
        state = {"busy": {"A": 0.0, "D": 0.0}, "pv_tiles": {}}

        def _inject_later(eng, ns):
            state["busy"][eng] += ns

        # ---- Phase P: q/k projections (fp8 DoubleRow, bias via 33rd
        # contraction row) -> PSUM f32 -> DMA straight into SBUF. No
        # element-wise conversion pass at all.
        stp_cm = tc.tile_pool(name="st", bufs=2, space="PSUM")
        stp = stp_cm.__enter__()  # closed explicitly before the final pool
        pp_cm = tc.tile_pool(name="proj_psum", bufs=1, space="PSUM")
        pp = pp_cm.__enter__()

        def qk_proj(qk, ch, eng):
            dst = qf if qk == 0 else kf
            ps = pp.tile([P, 512], f32, tag=f"ps{qk}", name=f"ps{qk}")
            nc.tensor.matmul(
                ps[:], wqk_sb[:, :, ds(128 * qk, P)], x_chunk(qk, ch),
                start=True, stop=True, perf_mode=DR)
            if eng == "A":
                nc.scalar.copy(dst[:, ts(ch, 512)], ps[:])
                _inject_later("A", 612)
            else:
                nc.vector.tensor_copy(dst[:, ts(ch, 512)], ps[:])
                _inject_later("D", 658)

        for (qk, ch, eng) in ((0, 0, "A"), (1, 0, "D"), (0, 1, "A"),
                              (0, 2, "D"), (0, 3, "A"), (1, 1, "D"),
                              (1, 2, "A"), (1, 3, "D")):
            qk_proj(qk, ch, eng)

        wl_sb = const.tile([P, NLT, OUT], bf16, tag="wl")
        nc.sync.dma_start(wl_sb[:], wl_t)

        def emit_vproj():
            # 4 l-tiles of V per [128,512] proj-pool tile (shared-bank
            # accumulation groups: one start covers the tile), one big
            # f32->fp8 copy each on ACT/DVE.
            for c4 in range(4):
                ps = pp.tile([P, 4, P], f32, tag=f"ps{c4 % 2}",
                             name="psv")
                for i in range(4):
                    nc.tensor.matmul(ps[:, i, :],
                                     vt_sb[:, ds(512 * c4 + 128 * i, P)],
                                     wv_sb[:],
                                     start=(i == 0), stop=(i == 3))
                dst = v2[:, ds(2 * c4, 2), :, :, :]
                if c4 % 2 == 0:
                    nc.scalar.copy(dst, ps[:])
                    _inject_later("A", 612)
                else:
                    nc.vector.tensor_copy(dst, ps[:])
                    _inject_later("D", 658)

        # ---- main pools --------------------------------------------------
        inject = _inject_later

        import os
        exppat = os.environ.get("EXPPAT", "AD" * 30 + "ADDA")
        state["tile_no"] = 0

        def pick_exp_engine():
            # Pattern override (tuning knob); else projected-finish greedy
            # constrained to never run >2 consecutive tiles on one engine
            # (the in-order PSUM ring needs a near-alternating cadence).
            n = state["tile_no"]
            state["tile_no"] = n + 1
            if exppat:
                eng = exppat[n % len(exppat)]
            else:
                a = state["busy"]["A"] + 1038.0
                d = state["busy"]["D"] + 1192.0
                eng = "A" if a <= d else "D"
                last2 = state.get("last2", "")
                if last2 == eng * 2:
                    eng = "D" if eng == "A" else "A"
                state["last2"] = (state.get("last2", "") + eng)[-2:]
            inject(eng, 1038.0 if eng == "A" else 1192.0)
            return eng

        def emit_scores(h, kcs, fillers=()):
            """per lt-pair (one engine): two score mms + ONE 1024-wide exp.
            fillers: PE-op callables interleaved between tiles to keep score
            production cadence even (B-phase mms would otherwise pause it)."""
            fillers = list(fillers)
            for kc in kcs:
                for ltp in range(8):
                    eng = pick_exp_engine()
                    st = stp.tile([P, 2, 4, P], f32, tag="st", name="st",
                                  bufs=3)
                    for j2 in range(2):
                        nc.tensor.matmul(
                            st[:, j2, :, :],
                            qf[ds(64 * h, 64), ts(2 * ltp + j2, P)],
                            kf[ds(64 * h, 64), ts(kc, KC)],
                            start=True, stop=True)
                    stv = st[:, :, :, :].transpose((0, 2, 1, 3))
                    out = et[:, h, ltp, ds(4 * kc, 4), :, :]
                    if eng == "A":
                        nc.scalar.activation(out, stv, AF.Exp, scale=0.125)
                    else:
                        nc.vector.tensor_scalar(
                            et_i8[:, h, ltp, ds(4 * kc, 4), :, :],
                            stv, SCH_A, SCH_B, op0=ALU.mult, op1=ALU.add)
                    if fillers:
                        f = fillers.pop(0)
                        if f is not None:
                            f()
            while fillers:
                f = fillers.pop(0)
                if f is not None:
                    f()

        def emit_z_mms(ktp):
            """Z matmuls (own accumulation group over pv[:,4,:])."""
            pv = state["pvp"].tile([P, 5, D], f32, tag="pv", name="pv")
            state["pv_tiles"][ktp] = pv
            for i in range(2):
                kt = 2 * ktp + i
                for h in range(2):
                    for ltp in range(8):
                        nc.tensor.matmul(
                            pv[:, 4, ds(2 * i + h, 1)],
                            et[:, h, ltp, kt, :, :],
                            ones2[:],
                            start=(i == 0 and h == 0 and ltp == 0),
                            stop=(i == 1 and h == 1 and ltp == 7),
                            perf_mode=DR)

        def emit_pv_mms(ktp):
            """PV matmuls (own group over pv[:,0:4,:])."""
            pv = state["pv_tiles"][ktp]
            for i in range(2):
                kt = 2 * ktp + i
                for h in range(2):
                    for ltp in range(8):
                        nc.tensor.matmul(
                            pv[:, 2 * i + h, :],
                            et[:, h, ltp, kt, :, :],
                            v2[:, ltp, :, h, :],
                            start=(i == 0 and h == 0 and ltp == 0),
                            stop=(i == 1 and h == 1 and ltp == 7),
                            perf_mode=DR)

        cmb_mode = os.environ.get("CMB", "pool")

        def emit_combines(ktp, mpool):
            # Per-ktp normalize + head-combine. Modes:
            #  dve:  recip/mult/add on DVE straight from PSUM (1x rate).
            #  pool: ACT copies pv->SBUF, Pool does mult/add.
            #  sbuf: ACT copies the WHOLE pv tile (PV+Z) -> SBUF bf16; DVE
            #        then runs recip/mult/add all-SBUF, where its 2x mode
            #        applies (~420ns/ktp vs ~780 PSUM-direct), and pv's
            #        PSUM bank frees at the copy.
            pv = state["pv_tiles"].pop(ktp)
            tmp = mpool.tile([P, 2, 2, D], f32, tag="tmp")
            rzb = rz[:, ds(4 * ktp, 4)].broadcast_to([P, 4, D])
            if cmb_mode == "sbuf":
                pvc = mpool.tile([P, 5, D], f32, tag="pvc")
                nc.scalar.copy(pvc[:], pv[:])
                inject("A", 452)
                nc.vector.reciprocal(rz[:, ds(4 * ktp, 4)], pvc[:, 4, 0:4])
                inject("D", 100)
                nc.vector.tensor_tensor(tmp[:], pvc[:, 0:4, :], rzb,
                                        op=ALU.mult)
                inject("D", 195)
                nc.vector.tensor_add(
                    m_sb[:, ds(2 * ktp, 2), :], tmp[:, :, 0, :],
                    tmp[:, :, 1, :])
                inject("D", 96)
                return
            nc.vector.reciprocal(rz[:, ds(4 * ktp, 4)], pv[:, 4, 0:4])
            inject("D", 129)
            if cmb_mode == "pool" and ktp < 6:
                pvc = mpool.tile([P, 2, 2, D], f32, tag="pvc")
                nc.scalar.copy(pvc[:], pv[:, 0:4, :])
                inject("A", 398)
                nc.gpsimd.tensor_tensor(tmp[:], pvc[:], rzb, op=ALU.mult)
                nc.gpsimd.tensor_add(
                    m_sb[:, ds(2 * ktp, 2), :], tmp[:, :, 0, :],
                    tmp[:, :, 1, :])
            else:
                nc.vector.tensor_tensor(tmp[:], pv[:, 0:4, :], rzb,
                                        op=ALU.mult)
                inject("D", 392)
                nc.vector.tensor_add(
                    m_sb[:, ds(2 * ktp, 2), :], tmp[:, :, 0, :],
                    tmp[:, :, 1, :])
                inject("D", 258)

        mpool = ctx.enter_context(tc.tile_pool(name="mp", bufs=2))

        # ---- pipeline ----------------------------------------------------
        emit_scores(0, [0])
        emit_vproj()
        emit_scores(1, [0])
        pp_cm.__exit__(None, None, None)

        pvp_cm = tc.tile_pool(name="pv", bufs=2, space="PSUM")
        state["pvp"] = pvp_cm.__enter__()

        emit_scores(0, [1])
        emit_scores(1, [1])

        # The TileScheduler orders by dependencies, not emission order, so
        # B/C work is simply emitted after each quarter's score blocks.
        emit_scores(0, [2])
        emit_scores(1, [2])
        for ktp in (0, 1):
            emit_z_mms(ktp)
            emit_pv_mms(ktp)
            emit_combines(ktp, mpool)
        emit_scores(0, [3])
        for ktp in (2, 3):
            emit_z_mms(ktp)
            emit_pv_mms(ktp)
            emit_combines(ktp, mpool)
        emit_scores(1, [3])
        for ktp in (4, 5, 6, 7):
            emit_z_mms(ktp)
            emit_pv_mms(ktp)
            emit_combines(ktp, mpool)

        pvp_cm.__exit__(None, None, None)
        stp_cm.__exit__(None, None, None)

        with tc.tile_pool(name="fop", bufs=1, space="PSUM") as fop:
            fo = fop.tile([P, 6 * D], f32, tag="fo", name="fo")
            # ONE accumulation group: a single start pends the whole-bank
            # zero region (a second start inside the bank re-pends it and
            # WIPES earlier groups' partial sums — measured 4e-3 absmax).
            for kt in range(NLT):
                for ob in range(6):
                    w = min(P, OUT - 128 * ob)
                    nc.tensor.matmul(
                        fo[0:w, ds(64 * ob, 64)],
                        wl_sb[:, kt, ds(128 * ob, w)],
                        m_sb[:, kt, :],
                        start=(kt == 0 and ob == 0),
                        stop=(kt == NLT - 1 and ob == 5))
            # copies split across engines; the copies finish staggered, so
            # two DMAs (first issued right after the ACT copy) beat one
            # DMA that must wait for both.
            nc.scalar.copy(fo_sb[:, 0:192], fo[:, 0:192])
            nc.sync.dma_start(f_out[:, 0:3, :], fo_sb[:, 0:192])
            nc.vector.tensor_copy(fo_sb[:, 192:384], fo[:, 192:384])
            nc.scalar.dma_start(f_out[:, 3:6, :], fo_sb[:, 192:384])

    nc.compile()
    _PROGRAM_CACHE["nc"] = nc
    return nc


def prep_in_maps(query, key, value, Wq, Wk, bk, Wv, Wl):
    """Host-side shard + layout prep: one in_map per core."""
    import ml_dtypes

    F8 = ml_dtypes.float8_e4m3
    BF = ml_dtypes.bfloat16

    query = np.asarray(query, np.float32)
    key = np.asarray(key, np.float32)
    value = np.asarray(value, np.float32)
    Wq = np.asarray(Wq, np.float32)
    Wk = np.asarray(Wk, np.float32)
    bk = np.asarray(bk, np.float32)
    Wv = np.asarray(Wv, np.float32)
    Wl = np.asarray(Wl, np.float32)

    wl_prep = np.ascontiguousarray(
        Wl.reshape(NLT, P, OUT).transpose(1, 0, 2).astype(BF))

    def dsplit33(xt, ones_row):
        # [64, L] -> [33, 2, L]: d = 32j + p for p<32; row 32 = (ones, 0)
        out = np.zeros((33, 2, xt.shape[1]), np.float32)
        out[:32] = xt.reshape(2, 32, -1).transpose(1, 0, 2)
        out[32, 0] = ones_row
        return np.ascontiguousarray(out)

    in_maps = []
    for c in range(N_CORES):
        b, g = divmod(c, 4)
        sl = slice(P * g, P * (g + 1))
        # wqk [33(p), 2(j), 2(qk), 128(o)]: row 32 j=0 = bias (k only)
        wqk = np.zeros((33, 2, 2, P), np.float32)
        wqk[:32, :, 0, :] = Wq[:, sl].reshape(2, 32, P).transpose(1, 0, 2)
        wqk[:32, :, 1, :] = Wk[:, sl].reshape(2, 32, P).transpose(1, 0, 2)
        wqk[32, 0, 1, :] = bk[sl]
        # qkx blob [33, 2, 256 wqk | 512 xq_c0 | 512 xk_c0 | 1536 xq_c1-3
        # | 1536 xk_c1-3]
        xq33 = dsplit33(query[b].T, 1.0).astype(F8)
        xk33 = dsplit33(key[b].T, 1.0).astype(F8)
        qkx = np.empty((33, 2, 4352), F8)
        qkx[:, :, 0:256] = wqk.astype(F8).reshape(33, 2, 256)
        qkx[:, :, 256:768] = xq33[:, :, 0:512]
        qkx[:, :, 768:1280] = xk33[:, :, 0:512]
        qkx[:, :, 1280:2816] = xq33[:, :, 512:]
        qkx[:, :, 2816:4352] = xk33[:, :, 512:]
        vtw = np.empty((D, L + P), BF)
        vtw[:, :L] = value[b].T.astype(BF)
        vtw[:, L:] = Wv[:, sl].astype(BF)
        in_maps.append({
            "qkx": qkx,
            "vtw": np.ascontiguousarray(vtw),
            "wl_t": wl_prep,
        })
    return in_maps


def combine_outputs(f_outs, bv, Wl, bl):
    """Host-side gather: per-core F^T blocks -> full output + biases."""
    bv = np.asarray(bv, np.float32)
    Wl = np.asarray(Wl, np.float32)
    bl = np.asarray(bl, np.float32)
    F = np.stack(f_outs).astype(np.float32)        # [8, 128, 6, 64]
    # [core, p, ob, d] -> [core, d, 128*ob + p] -> F [core, 64, 720]
    Fc = F.transpose(0, 3, 2, 1).reshape(N_CORES, D, 6 * P)[:, :, :OUT]
    out = np.empty((B, D, OUT), np.float32)
    for b in range(B):
        out[b] = 0.125 * Fc[4 * b:4 * b + 4].sum(axis=0)
    bv_mean = bv.reshape(H, D).mean(axis=0)
    out += bv_mean[None, :, None] * Wl.sum(axis=0)[None, None, :]
    out += bl[None, None, :]
    return out


def kernel(query, key, value, Wq, bq, Wk, bk, Wv, bv, Wl, bl):
    from concourse.bass_utils import run_bass_kernel_spmd

    nc = build_program()
    in_maps = prep_in_maps(query, key, value, Wq, Wk, bk, Wv, Wl)
    res = run_bass_kernel_spmd(nc, in_maps, core_ids=list(range(N_CORES)))
    f_outs = [res.results[c]["f_out"] for c in range(N_CORES)]
    return combine_outputs(f_outs, bv, Wl, bl)



# revision 57
# speedup vs baseline: 1.0081x; 1.0073x over previous
"""Trainium2 Bass kernel for nn_CrossAttention_77240691851613.

Reference (B=2, L=2048, D=64, H=8, OUT=720), per core (batch b, 2 heads):
    q = x_q @ Wq          k = x_k @ Wk + bk      v = x_v @ Wv
    S^T[l,k] = q^T . k^T  (contraction d=64)     P = exp(S^T/8)
    out[k,d] = sum_l P[l,k] V[l,d] / Z[k],  Z = sum_l P
    F = mean_h(out)^T @ Wl  (+ biases on host)

Design (sharding: 8 cores = 2 batches x 4 head-groups of 2 heads):
 - Inputs ship as 3 blob DMAs (wqk+xq/xk chunk0 head, rest, vt+wv);
   each extra dma_start costs ~625ns HWDGE + 900ns sem serially at
   startup, and the head blob lets proj->copy->score->exp start ~2us
   earlier. wl (2.8MB) loads mid-stream.
 - q/k projections in fp8e4 + DoubleRow (0.5 cyc/row); bk is folded in
   as a 33rd contraction row (fp8 bias/ones row). Scores are bf16.
 - exp of 8.4M scores is the wall: [128,1024] lt-pair tiles alternate
   strictly between ACT (true exp -> fp8e4 out, 1038ns) and DVE
   (Schraudolph bit trick: i8 = round(S*1.4427) + 56 IS the e4m3 bit
   pattern of exp(S/8); +-4% rel err averages out under softmax).
   GPSIMD cannot read PSUM on HW, so only these 2 engines. The st ring
   (bufs=3) locksteps the pair cadence at ~1.34us (ring-lag ~900ns /
   3 bufs); strict AD alternation measured best by scan.
 - et layout [128, h, ltp, kt, j2, 128] keeps every exp write and
   every PV read a tight contiguous box (no phantom overlap deps).
 - PV flipped + fp8 DoubleRow over l-tile pairs: stationary et
   [128,2,128], moving V [128,2,64] -> out [k=128,64]; Z via 1-col
   ones matmuls into pv[:,4,:] (own group). Per kt-pair combine:
   DVE reciprocal, then ACT copies pv->SBUF f32 and the otherwise-idle
   GPSIMD/Pool does the broadcast multiply (pv[j] * rz[j]) + strided
   pair-add into m (takes ~6us of combine work off the two hot
   engines); the tail ktps 6,7 stay on DVE (free at that point).
   NOTE: the TileScheduler orders by deps, not emission order.
 - final projection flipped: out [720-block, 64], moving = m (64
   cols), all 6 blocks accumulating in ONE psum bank with a SINGLE
   start (a second start in the same bank re-pends the whole-bank
   zero region and wipes earlier partial sums - cost 4e-3 absmax).
 - single output DMA; fo copies split ACT/DVE.
 - bq cancels in softmax over l; bv and the head-mean 1/8 are applied
   on the host gather path.
"""

import numpy as np

B = 2
L = 2048
D = 64
H = 8
OUT = 720
P = 128
KC = 512  # score k-chunk (one PSUM bank)
NLT = 16
N_CORES = 8

# e4m3-bit-trick exp constants: i8 = round(S * (0.125*8*log2e)) + 56
SCH_A = 1.4426950408889634
SCH_B = 56.0

_PROGRAM_CACHE = {}


def build_program():
    if "nc" in _PROGRAM_CACHE:
        return _PROGRAM_CACHE["nc"]

    from contextlib import ExitStack

    import concourse.bass as bass
    import concourse.tile as tile
    from concourse import bacc, mybir

    dt = mybir.dt
    f32 = dt.float32
    bf16 = dt.bfloat16
    f8 = dt.float8e4
    i8 = dt.int8
    AF = mybir.ActivationFunctionType
    ALU = mybir.AluOpType
    DR = mybir.MatmulPerfMode.DoubleRow
    ts = bass.ts
    ds = bass.ds

    nc = bacc.Bacc("TRN2", target_bir_lowering=False, debug=False,
                   num_devices=N_CORES)

    # ---- DRAM I/O --------------------------------------------------------
    # qkx blob [33, 2(j), 256 wqk | 512 xq_c0 | 512 xk_c0 | 1536 xq_c1-3
    # | 1536 xk_c1-3] f8. Two DMAs: a small head (wqk + chunk 0 of q/k)
    # lands ~1.6us so the first proj->copy->score->exp chain starts ~3us
    # earlier than one monolithic load; the rest follows right behind.
    qkx_t = nc.dram_tensor("qkx", [33, 2, 4352], f8, kind="ExternalInput").ap()
    # vtw blob [64, 2048 vt | 128 wv] bf16.
    vtw_t = nc.dram_tensor("vtw", [D, L + P], bf16, kind="ExternalInput").ap()
    wl_t = nc.dram_tensor("wl_t", [P, NLT, OUT], bf16, kind="ExternalInput").ap()
    f_out = nc.dram_tensor("f_out", [P, 6, D], f32, kind="ExternalOutput").ap()

    with tile.TileContext(nc) as tc, ExitStack() as ctx:
        const = ctx.enter_context(tc.tile_pool(name="const", bufs=1))

        # ---- SBUF persistent tiles --------------------------------------
        qkx_sb = const.tile([33, 2, 4352], f8, tag="qkx")
        nc.sync.dma_start(qkx_sb[:, :, 0:1280], qkx_t[:, :, 0:1280])
        nc.sync.dma_start(qkx_sb[:, :, ds(1280, 3072)],
                          qkx_t[:, :, ds(1280, 3072)])
        vtw_sb = const.tile([D, L + P], bf16, tag="vtw")
        nc.sync.dma_start(vtw_sb[:], vtw_t)
        wqk_sb = qkx_sb[:, :, 0:256]          # [33, 2, 2*128]: qk via ds()
        vt_sb = vtw_sb[:, 0:L]
        wv_sb = vtw_sb[:, ds(L, P)]

        def x_chunk(qk, ch):
            # 512-col l-chunk of xq (qk=0) / xk (qk=1) in the blob
            off = 256 + 512 * qk if ch == 0 else 1280 + 1536 * qk + 512 * (ch - 1)
            return qkx_sb[:, :, ds(off, 512)]

        # q/k [128(h,d), L] bf16, filled chunkwise by f32->bf16 copies
        # spread across engines (the only transport PSUM allows).
        qf = const.tile([P, L], bf16, tag="qf")
        kf = const.tile([P, L], bf16, tag="kf")
        # exp tile [P, h, ltp, kt(16), j2(2), 128]: PV lhsT (h,ltp,kt) reads
        # the contiguous 256B window; exp writes (h,lt,kc) cover 4 kt
        # sub-windows of one j2 — boxes never span other k-quarters.
        et = const.tile([P, 2, 8, NLT, 2, P], f8, tag="et")
        et_i8 = et.bitcast(i8)
        # v2 (ltp, j2, h, d + ones-col 64): PV matmuls' 65th moving col
        # accumulates Z = sum_l P directly -- no separate Z matmuls.
        v2 = const.tile([P, 8, 2, 2, D + 1], f8, tag="v2")
        rz = const.tile([P, 32], f32, tag="rz")         # 1/Z, col=(kt,h)
        m_sb = const.tile([P, NLT, D], bf16, tag="m")   # combined heads
        fo_sb = const.tile([P, 6 * D], f32, tag="fo")

        # ACT exp-table warmup
        warm = const.tile([1, 8], f32, tag="warm")
        nc.vector.memset(warm[:], 0.0)
        nc.scalar.activation(warm[:], warm[:], AF.Exp)
        nc.gpsimd.memset(v2[:, :, :, :, ds(D, 1)], 1.0)

        # Greedy (projected-finish) engine-assignment bookkeeping: every
        # elementwise op injects its modeled busy cost (TimelineSim: ACT
        # 185ns + 0.833/col, DVE 125ns + 1.042/col) into its engine total.
        state = {"busy": {"A": 0.0, "D": 0.0}, "pv_tiles": {}}

        def _inject_later(eng, ns):
            state["busy"][eng] += ns

        # ---- Phase P: q/k projections (fp8 DoubleRow, bias via 33rd
        # contraction row) -> PSUM f32 -> DMA straight into SBUF. No
        # element-wise conversion pass at all.
        stp_cm = tc.tile_pool(name="st", bufs=2, space="PSUM")
        stp = stp_cm.__enter__()  # closed explicitly before the final pool
        pp_cm = tc.tile_pool(name="proj_psum", bufs=1, space="PSUM")
        pp = pp_cm.__enter__()

        def qk_proj(qk, ch, eng):
            dst = qf if qk == 0 else kf
            ps = pp.tile([P, 512], f32, tag=f"ps{qk}", name=f"ps{qk}")
            nc.tensor.matmul(
                ps[:], wqk_sb[:, :, ds(128 * qk, P)], x_chunk(qk, ch),
                start=True, stop=True, perf_mode=DR)
            if eng == "A":
                nc.scalar.copy(dst[:, ts(ch, 512)], ps[:])
                _inject_later("A", 612)
            else:
                nc.vector.tensor_copy(dst[:, ts(ch, 512)], ps[:])
                _inject_later("D", 658)

        for (qk, ch, eng) in ((0, 0, "A"), (1, 0, "D"), (0, 1, "A"),
                              (0, 2, "D"), (0, 3, "A"), (1, 1, "D"),
                              (1, 2, "A"), (1, 3, "D")):
            qk_proj(qk, ch, eng)

        wl_sb = const.tile([P, NLT, OUT], bf16, tag="wl")
        nc.sync.dma_start(wl_sb[:], wl_t)

        def emit_vproj():
            # 4 l-tiles of V per [128,512] proj-pool tile (shared-bank
            # accumulation groups: one start covers the tile), one big
            # f32->fp8 copy each on ACT/DVE.
            for c4 in range(4):
                ps = pp.tile([P, 4, P], f32, tag=f"ps{c4 % 2}",
                             name="psv")
                for i in range(4):
                    nc.tensor.matmul(ps[:, i, :],
                                     vt_sb[:, ds(512 * c4 + 128 * i, P)],
                                     wv_sb[:],
                                     start=(i == 0), stop=(i == 3))
                dst = v2[:, ds(2 * c4, 2), :, :, 0:D]
                if c4 % 2 == 0:
                    nc.scalar.copy(dst, ps[:])
                    _inject_later("A", 612)
                else:
                    nc.vector.tensor_copy(dst, ps[:])
                    _inject_later("D", 658)

        # ---- main pools --------------------------------------------------
        inject = _inject_later

        import os
        exppat = os.environ.get("EXPPAT", "AD" * 30 + "ADDA")
        state["tile_no"] = 0

        def pick_exp_engine():
            # Pattern override (tuning knob); else projected-finish greedy
            # constrained to never run >2 consecutive tiles on one engine
            # (the in-order PSUM ring needs a near-alternating cadence).
            n = state["tile_no"]
            state["tile_no"] = n + 1
            if exppat:
                eng = exppat[n % len(exppat)]
            else:
                a = state["busy"]["A"] + 1038.0
                d = state["busy"]["D"] + 1192.0
                eng = "A" if a <= d else "D"
                last2 = state.get("last2", "")
                if last2 == eng * 2:
                    eng = "D" if eng == "A" else "A"
                state["last2"] = (state.get("last2", "") + eng)[-2:]
            inject(eng, 1038.0 if eng == "A" else 1192.0)
            return eng

        def emit_scores(h, kcs, fillers=()):
            """per lt-pair (one engine): two score mms + ONE 1024-wide exp.
            fillers: PE-op callables interleaved between tiles to keep score
            production cadence even (B-phase mms would otherwise pause it)."""
            fillers = list(fillers)
            for kc in kcs:
                for ltp in range(8):
                    eng = pick_exp_engine()
                    st = stp.tile([P, 2, 4, P], f32, tag="st", name="st",
                                  bufs=3)
                    for j2 in range(2):
                        nc.tensor.matmul(
                            st[:, j2, :, :],
                            qf[ds(64 * h, 64), ts(2 * ltp + j2, P)],
                            kf[ds(64 * h, 64), ts(kc, KC)],
                            start=True, stop=True)
                    stv = st[:, :, :, :].transpose((0, 2, 1, 3))
                    out = et[:, h, ltp, ds(4 * kc, 4), :, :]
                    if eng == "A":
                        nc.scalar.activation(out, stv, AF.Exp, scale=0.125)
                    else:
                        nc.vector.tensor_scalar(
                            et_i8[:, h, ltp, ds(4 * kc, 4), :, :],
                            stv, SCH_A, SCH_B, op0=ALU.mult, op1=ALU.add)
                    if fillers:
                        f = fillers.pop(0)
                        if f is not None:
                            f()
            while fillers:
                f = fillers.pop(0)
                if f is not None:
                    f()

        def emit_pv_mms(ktp):
            """PV matmuls; moving col 64 (ones) accumulates Z in-place."""
            pv = state["pvp"].tile([P, 4, D + 1], f32, tag="pv", name="pv")
            state["pv_tiles"][ktp] = pv
            for i in range(2):
                kt = 2 * ktp + i
                for h in range(2):
                    for ltp in range(8):
                        nc.tensor.matmul(
                            pv[:, 2 * i + h, :],
                            et[:, h, ltp, kt, :, :],
                            v2[:, ltp, :, h, :],
                            start=(i == 0 and h == 0 and ltp == 0),
                            stop=(i == 1 and h == 1 and ltp == 7),
                            perf_mode=DR)

        cmb_mode = os.environ.get("CMB", "pool")

        def emit_combines(ktp, mpool):
            # Per-ktp normalize + head-combine. Modes:
            #  dve:  recip/mult/add on DVE straight from PSUM (1x rate).
            #  pool: ACT copies pv->SBUF, Pool does mult/add.
            #  sbuf: ACT copies the WHOLE pv tile (PV+Z) -> SBUF bf16; DVE
            #        then runs recip/mult/add all-SBUF, where its 2x mode
            #        applies (~420ns/ktp vs ~780 PSUM-direct), and pv's
            #        PSUM bank frees at the copy.
            pv = state["pv_tiles"].pop(ktp)
            tmp = mpool.tile([P, 2, 2, D], f32, tag="tmp")
            rzb = rz[:, ds(4 * ktp, 4)].broadcast_to([P, 4, D])
            if cmb_mode == "sbuf" or (cmb_mode == "pool" and ktp == 7):
                # tail ktp7: ACT's exact f32 copy overlaps ktp6's DVE
                # chain; DVE then runs recip/mult/add all-SBUF at 2x.
                pvc = mpool.tile([P, 4, D + 1], f32, tag="pvc")
                nc.scalar.copy(pvc[:], pv[:])
                inject("A", 452)
                nc.vector.reciprocal(rz[:, ds(4 * ktp, 4)],
                                     pvc[:, :, ds(D, 1)])
                inject("D", 100)
                nc.vector.tensor_tensor(tmp[:], pvc[:, :, 0:D], rzb,
                                        op=ALU.mult)
                inject("D", 195)
                nc.vector.tensor_add(
                    m_sb[:, ds(2 * ktp, 2), :], tmp[:, :, 0, :],
                    tmp[:, :, 1, :])
                inject("D", 96)
                return
            nc.vector.reciprocal(rz[:, ds(4 * ktp, 4)], pv[:, :, ds(D, 1)])
            inject("D", 129)
            if cmb_mode == "pool" and ktp < 6:
                pvc = mpool.tile([P, 2, 2, D], f32, tag="pvc")
                nc.scalar.copy(pvc[:], pv[:, :, 0:D])
                inject("A", 398)
                nc.gpsimd.tensor_tensor(tmp[:], pvc[:], rzb, op=ALU.mult)
                nc.gpsimd.tensor_add(
                    m_sb[:, ds(2 * ktp, 2), :], tmp[:, :, 0, :],
                    tmp[:, :, 1, :])
            else:
                nc.vector.tensor_tensor(tmp[:], pv[:, :, 0:D], rzb,
                                        op=ALU.mult)
                inject("D", 392)
                nc.vector.tensor_add(
                    m_sb[:, ds(2 * ktp, 2), :], tmp[:, :, 0, :],
                    tmp[:, :, 1, :])
                inject("D", 258)

        mpool = ctx.enter_context(tc.tile_pool(name="mp", bufs=2))

        # ---- pipeline ----------------------------------------------------
        emit_scores(0, [0])
        emit_vproj()
        emit_scores(1, [0])
        pp_cm.__exit__(None, None, None)

        pvp_cm = tc.tile_pool(name="pv", bufs=2, space="PSUM")
        state["pvp"] = pvp_cm.__enter__()

        emit_scores(0, [1])
        emit_scores(1, [1])

        # The TileScheduler orders by dependencies, not emission order, so
        # B/C work is simply emitted after each quarter's score blocks.
        emit_scores(0, [2])
        emit_scores(1, [2])
        for ktp in (0, 1):
            emit_pv_mms(ktp)
            emit_combines(ktp, mpool)
        emit_scores(0, [3])
        for ktp in (2, 3):
            emit_pv_mms(ktp)
            emit_combines(ktp, mpool)
        emit_scores(1, [3])
        for ktp in (4, 5, 6, 7):
            emit_pv_mms(ktp)
            emit_combines(ktp, mpool)

        pvp_cm.__exit__(None, None, None)
        stp_cm.__exit__(None, None, None)

        with tc.tile_pool(name="fop", bufs=1, space="PSUM") as fop:
            fo = fop.tile([P, 6 * D], f32, tag="fo", name="fo")
            # ONE accumulation group: a single start pends the whole-bank
            # zero region (a second start inside the bank re-pends it and
            # WIPES earlier groups' partial sums — measured 4e-3 absmax).
            for kt in range(NLT):
                for ob in range(6):
                    w = min(P, OUT - 128 * ob)
                    nc.tensor.matmul(
                        fo[0:w, ds(64 * ob, 64)],
                        wl_sb[:, kt, ds(128 * ob, w)],
                        m_sb[:, kt, :],
                        start=(kt == 0 and ob == 0),
                        stop=(kt == NLT - 1 and ob == 5))
            # copies split across engines; staggered finish -> two DMAs
            nc.scalar.copy(fo_sb[:, 0:192], fo[:, 0:192])
            nc.sync.dma_start(f_out[:, 0:3, :], fo_sb[:, 0:192])
            nc.vector.tensor_copy(fo_sb[:, 192:384], fo[:, 192:384])
            nc.scalar.dma_start(f_out[:, 3:6, :], fo_sb[:, 192:384])

    nc.compile()
    _PROGRAM_CACHE["nc"] = nc
    return nc


def prep_in_maps(query, key, value, Wq, Wk, bk, Wv, Wl):
    """Host-side shard + layout prep: one in_map per core."""
    import ml_dtypes

    F8 = ml_dtypes.float8_e4m3
    BF = ml_dtypes.bfloat16

    query = np.asarray(query, np.float32)
    key = np.asarray(key, np.float32)
    value = np.asarray(value, np.float32)
    Wq = np.asarray(Wq, np.float32)
    Wk = np.asarray(Wk, np.float32)
    bk = np.asarray(bk, np.float32)
    Wv = np.asarray(Wv, np.float32)
    Wl = np.asarray(Wl, np.float32)

    wl_prep = np.ascontiguousarray(
        Wl.reshape(NLT, P, OUT).transpose(1, 0, 2).astype(BF))

    def dsplit33(xt, ones_row):
        # [64, L] -> [33, 2, L]: d = 32j + p for p<32; row 32 = (ones, 0)
        out = np.zeros((33, 2, xt.shape[1]), np.float32)
        out[:32] = xt.reshape(2, 32, -1).transpose(1, 0, 2)
        out[32, 0] = ones_row
        return np.ascontiguousarray(out)

    in_maps = []
    for c in range(N_CORES):
        b, g = divmod(c, 4)
        sl = slice(P * g, P * (g + 1))
        # wqk [33(p), 2(j), 2(qk), 128(o)]: row 32 j=0 = bias (k only)
        wqk = np.zeros((33, 2, 2, P), np.float32)
        wqk[:32, :, 0, :] = Wq[:, sl].reshape(2, 32, P).transpose(1, 0, 2)
        wqk[:32, :, 1, :] = Wk[:, sl].reshape(2, 32, P).transpose(1, 0, 2)
        wqk[32, 0, 1, :] = bk[sl]
        # qkx blob [33, 2, 256 wqk | 512 xq_c0 | 512 xk_c0 | 1536 xq_c1-3
        # | 1536 xk_c1-3]
        xq33 = dsplit33(query[b].T, 1.0).astype(F8)
        xk33 = dsplit33(key[b].T, 1.0).astype(F8)
        qkx = np.empty((33, 2, 4352), F8)
        qkx[:, :, 0:256] = wqk.astype(F8).reshape(33, 2, 256)
        qkx[:, :, 256:768] = xq33[:, :, 0:512]
        qkx[:, :, 768:1280] = xk33[:, :, 0:512]
        qkx[:, :, 1280:2816] = xq33[:, :, 512:]
        qkx[:, :, 2816:4352] = xk33[:, :, 512:]
        vtw = np.empty((D, L + P), BF)
        vtw[:, :L] = value[b].T.astype(BF)
        vtw[:, L:] = Wv[:, sl].astype(BF)
        in_maps.append({
            "qkx": qkx,
            "vtw": np.ascontiguousarray(vtw),
            "wl_t": wl_prep,
        })
    return in_maps


def combine_outputs(f_outs, bv, Wl, bl):
    """Host-side gather: per-core F^T blocks -> full output + biases."""
    bv = np.asarray(bv, np.float32)
    Wl = np.asarray(Wl, np.float32)
    bl = np.asarray(bl, np.float32)
    F = np.stack(f_outs).astype(np.float32)        # [8, 128, 6, 64]
    # [core, p, ob, d] -> [core, d, 128*ob + p] -> F [core, 64, 720]
    Fc = F.transpose(0, 3, 2, 1).reshape(N_CORES, D, 6 * P)[:, :, :OUT]
    out = np.empty((B, D, OUT), np.float32)
    for b in range(B):
        out[b] = 0.125 * Fc[4 * b:4 * b + 4].sum(axis=0)
    bv_mean = bv.reshape(H, D).mean(axis=0)
    out += bv_mean[None, :, None] * Wl.sum(axis=0)[None, None, :]
    out += bl[None, None, :]
    return out


def kernel(query, key, value, Wq, bq, Wk, bk, Wv, bv, Wl, bl):
    from concourse.bass_utils import run_bass_kernel_spmd

    nc = build_program()
    in_maps = prep_in_maps(query, key, value, Wq, Wk, bk, Wv, Wl)
    res = run_bass_kernel_spmd(nc, in_maps, core_ids=list(range(N_CORES)))
    f_outs = [res.results[c]["f_out"] for c in range(N_CORES)]
    return combine_outputs(f_outs, bv, Wl, bl)

